# revision 1
# baseline (speedup 1.0000x reference)
"""Binary-weight 3x3 conv (BinaryConv2d) Trainium2 Bass kernel.

Reference computation (for x[32,256,56,56] f32, w[256,256,3,3] f32, b[256] f32):
    out = conv2d(x, sign(w), pad=1) + sign(b)[None,:,None,None]

Strategy:
  - Data-parallel over batch: 8 cores x 4 images each. No collectives.
  - Conv expressed as 9 shifted matmuls (taps) over a zero-padded SBUF image,
    contracting input channels (2 chunks of 128) into PSUM.
  - x is split into bf16 hi + lo (lo = x - hi, exact by Sterbenz); both passes
    accumulate -> fp32-grade accuracy at bf16 PE throughput (weights are
    exactly +-1 in bf16/fp8).
  - mode "fp8lo": the lo pass runs in fp8e4m3 with perf_mode=DoubleRow, which
    contracts both 128-channel chunks in one matmul (9 MMs instead of 18 for
    the lo pass). lo is pre-scaled by 512 so values sit in e4m3's normal
    range; the combine step scales back by 1/512.
  - Weights are binarized on-chip (ACT Sign) and transposed per-tap via the PE
    transpose path into [in_ch, out_ch] stationary tiles.
  - Output: PSUM -> SBUF with per-partition binarized bias, then DMA out.
"""

from contextlib import ExitStack

import numpy as np

import concourse.bacc as bacc
import concourse.bass as bass
import concourse.tile as tile
import concourse.mybir as mybir
from concourse import masks
from concourse.bass_utils import run_bass_kernel_spmd

F32 = mybir.dt.float32
BF16 = mybir.dt.bfloat16
FP8 = mybir.dt.float8e4

N_CORES = 8
B, C, H, W = 32, 256, 56, 56
O = 256
KH = KW = 3
BPC = B // N_CORES  # images per core

ROWS_PER_TILE = 8          # output rows per PSUM tile (8*56 = 448 <= 512 free)
KI = C // 128              # input-channel chunks (contraction)
OC = O // 128              # output-channel chunks
LO_SCALE = 512.0           # fp8 lo pre-scale (2^9, exact in fp)

# "f32r": single-pass fp32r matmuls — 1 PE cycle/row at N>=256 (same rate as
# bf16) with ~19-bit effective input mantissa on TRN2 silicon. Measured on HW:
# 213 us/iter, scale-relative absmax 1.04e-4 vs the fp32 reference.
# "bf16": hi/lo two-pass — 397 us/iter, 2.7e-6 (use if a tighter-than-1e-4
# accuracy gate is ever required).
MODE = "f32r"              # "bf16" | "fp8lo" | "f32r"


def build_program(bpc=BPC, h=H, w=W, repeat=1, mode=None):
    """Build the per-core Bass program. Returns compiled nc."""
    mode = MODE if mode is None else mode
    ph, pw = h + 2, w + 4
    n_row_chunks = max(1, h // ROWS_PER_TILE)
    rows = h // n_row_chunks

    nc = bacc.Bacc("TRN2", target_bir_lowering=False, debug=False,
                   num_devices=N_CORES)
    x_d = nc.dram_tensor("x", [bpc, C, h, w], F32, kind="ExternalInput").ap()
    w_d = nc.dram_tensor("weight", [O, C, KH, KW], F32,
                         kind="ExternalInput").ap()
    b_d = nc.dram_tensor("bias", [O], F32, kind="ExternalInput").ap()
    o_d = nc.dram_tensor("out", [bpc, O, h, w], F32, kind="ExternalOutput").ap()

    with tile.TileContext(nc) as tc, ExitStack() as ctx:
        const = ctx.enter_context(tc.tile_pool(name="const", bufs=1))
        wstg_p = ctx.enter_context(tc.tile_pool(name="wstg", bufs=2))
        xstg_p = ctx.enter_context(tc.tile_pool(name="xstg", bufs=5))
        hif_p = ctx.enter_context(tc.tile_pool(name="hif", bufs=2))
        xpad_p = ctx.enter_context(tc.tile_pool(name="xpad", bufs=2))
        out_p = ctx.enter_context(tc.tile_pool(name="outp", bufs=4))
        npsA = 6 if mode == "bf16" else 3

        # ---- constants ----
        # f32r mode: both matmul operands must be f32r (walrus rejects mixed
        # 32-bit/non-32-bit); weights are engine-rounded to f32r (+-1 exact).
        F32R = mybir.dt.float32r
        wdt = F32 if mode == "f32r" else BF16
        ldt = F32R if mode == "f32r" else BF16
        identity = const.tile([128, 128], wdt)
        masks.make_identity(nc, identity[:])

        bias_raw = const.tile([128, OC], F32)
        bias_bin = const.tile([128, OC], F32)
        # bias_raw[p, oc] = bias[oc*128 + p]
        nc.sync.dma_start(out=bias_raw[:],
                          in_=b_d.rearrange("(b a) -> a b", b=OC))
        nc.scalar.sign(bias_bin[:], bias_raw[:])

        # ---- weights: load, binarize, transpose per tap ----
        # lhsT_all[:, idx, :] = sign(W[oc_chunk, ki_chunk, tap]).T  (shape [i,o])
        lhsT_all = const.tile([128, KI * KH * KW * OC, 128], ldt)

        def lidx(ki, ky, kx, oc):
            return ((ki * KH + ky) * KW + kx) * OC + oc

        tpsum_ctx = ExitStack()
        tpsum_p = tpsum_ctx.enter_context(
            tc.tile_pool(name="tpsum", bufs=2, space=bass.MemorySpace.PSUM))
        for ki in range(KI):
            for oc in range(OC):
                wstg = wstg_p.tile([128, 128, KH, KW], F32, tag="wstg")
                nc.sync.dma_start(
                    out=wstg[:],
                    in_=w_d[oc * 128:(oc + 1) * 128, ki * 128:(ki + 1) * 128, :, :])
                wbin = wstg_p.tile([128, 128, KH, KW], wdt, tag="wbin")
                nc.scalar.sign(wbin[:], wstg[:])
                for ky in range(KH):
                    for kx in range(KW):
                        tp = tpsum_p.tile([128, 128], wdt)
                        nc.tensor.transpose(tp[:], wbin[:, :, ky, kx], identity[:])
                        nc.vector.tensor_copy(
                            lhsT_all[:, lidx(ki, ky, kx, oc), :], tp[:])

        if mode == "fp8lo":
            # lhsT8[:, j, ki, :] with j = (ky*KW+kx)*OC+oc : fp8 copies of the
            # per-tap transposed weights, ki-chunks adjacent for DoubleRow.
            lhsT8 = const.tile([128, KH * KW * OC, KI, 128], FP8)
            for ki in range(KI):
                for oc in range(OC):
                    for ky in range(KH):
                        for kx in range(KW):
                            j = (ky * KW + kx) * OC + oc
                            nc.vector.tensor_copy(
                                lhsT8[:, j, ki, :],
                                lhsT_all[:, lidx(ki, ky, kx, oc), :])

        tpsum_ctx.close()
        psum_p = ctx.enter_context(
            tc.tile_pool(name="psum", bufs=npsA, space=bass.MemorySpace.PSUM))
        if mode == "fp8lo":
            psumB_p = ctx.enter_context(
                tc.tile_pool(name="psumB", bufs=3, space=bass.MemorySpace.PSUM))

        # ---- main loop over images ----
        for _rep in range(repeat):
            for n in range(bpc):
                xpad = {}
                lo8 = None
                if mode == "fp8lo":
                    # Flat 57-pitch wrap layout per chunk: buffer index of
                    # x[r, c] is 1 + (r+1)*57 + c; the zero column at c=56 of
                    # each row doubles as right pad of row r and (via wrap)
                    # left pad of row r+1. Leading/trailing 57-blocks are the
                    # vertical zero rows. DoubleRow rhs slices are 3-D
                    # [128, KI, 8*57] contiguous per chunk.
                    fw = w + 1
                    flat = (h + 2) * fw + 2
                    flat_pad = -flat % 16
                    lo8 = xpad_p.tile([128, KI, flat + flat_pad], FP8,
                                      tag="lo8")
                    nc.gpsimd.memset(lo8[:, :, 0:fw + 1], 0.0)
                    nc.gpsimd.memset(lo8[:, :, (h + 1) * fw + 1:], 0.0)
                    for ki in range(KI):
                        body = lo8[:, ki, fw + 1:(h + 1) * fw + 1].rearrange(
                            "p (r c) -> p r c", c=fw)
                        nc.gpsimd.memset(body[:, :, w:fw], 0.0)
                if mode == "f32r":
                    # Single-pass fp32r: x is rounded to f32r by an ACT copy
                    # into the padded tile (the BIR verifier requires f32r
                    # matmul inputs to be engine-rounded, not raw DMA).
                    for ki in range(KI):
                        xf = xstg_p.tile([128, h, w], F32, tag="xf")
                        # two half-loads -> two DMA queues in parallel
                        hh = h // 2
                        nc.sync.dma_start(
                            out=xf[:, :hh, :],
                            in_=x_d[n, ki * 128:(ki + 1) * 128, :hh, :])
                        nc.sync.dma_start(
                            out=xf[:, hh:, :],
                            in_=x_d[n, ki * 128:(ki + 1) * 128, hh:, :])
                        xp = xpad_p.tile([128, ph, pw], F32R, tag=f"x{ki}")
                        xpf = xp[:].bitcast(F32)
                        nc.gpsimd.memset(xpf[:, 0, :], 0.0)
                        nc.gpsimd.memset(xpf[:, ph - 1, :], 0.0)
                        nc.gpsimd.memset(xpf[:, 1:ph - 1, 0], 0.0)
                        nc.gpsimd.memset(xpf[:, 1:ph - 1, w + 1:pw], 0.0)
                        for rc in range(n_row_chunks):
                            a, b = rc * rows, rc * rows + rows
                            nc.scalar.copy(xp[:, 1 + a:1 + b, 1:w + 1],
                                           xf[:, a:b, :])
                        xpad[("hi", ki)] = xp
                    for rc in range(n_row_chunks):
                        for oc in range(OC):
                            r0 = rc * rows
                            ps = psum_p.tile([128, rows, w], F32)
                            k = 0
                            nmm = KI * KH * KW
                            for ki in range(KI):
                                xp = xpad[("hi", ki)]
                                for ky in range(KH):
                                    for kx in range(KW):
                                        nc.tensor.matmul(
                                            ps[:],
                                            lhsT_all[:, lidx(ki, ky, kx, oc), :],
                                            xp[:, r0 + ky:r0 + ky + rows,
                                               kx:kx + w],
                                            start=(k == 0),
                                            stop=(k == nmm - 1))
                                        k += 1
                            ob = out_p.tile([128, rows, w], F32)
                            nc.scalar.activation(
                                ob[:], ps[:],
                                mybir.ActivationFunctionType.Identity,
                                bias=bias_bin[:, oc:oc + 1], scale=1.0)
                            nc.sync.dma_start(
                                out=o_d[n, oc * 128:(oc + 1) * 128,
                                        r0:r0 + rows, :],
                                in_=ob[:])
                    continue
                for ki in range(KI):
                    xf = xstg_p.tile([128, h, w], F32, tag="xf")
                    nc.sync.dma_start(out=xf[:],
                                      in_=x_d[n, ki * 128:(ki + 1) * 128, :, :])
                    hi = xpad_p.tile([128, ph, pw], BF16, tag=f"hi{ki}")
                    nc.gpsimd.memset(hi[:, 0, :], 0.0)
                    nc.gpsimd.memset(hi[:, ph - 1, :], 0.0)
                    nc.gpsimd.memset(hi[:, 1:ph - 1, 0], 0.0)
                    nc.gpsimd.memset(hi[:, 1:ph - 1, w + 1:pw], 0.0)
                    xpad[("hi", ki)] = hi
                    if mode == "bf16":
                        lo = xpad_p.tile([128, ph, pw], BF16, tag=f"lo{ki}")
                        nc.gpsimd.memset(lo[:, 0, :], 0.0)
                        nc.gpsimd.memset(lo[:, ph - 1, :], 0.0)
                        nc.gpsimd.memset(lo[:, 1:ph - 1, 0], 0.0)
                        nc.gpsimd.memset(lo[:, 1:ph - 1, w + 1:pw], 0.0)
                        # Chunked by row group so downstream matmuls can start
                        # before the whole image is converted, and so PSUM
                        # drains never queue behind a multi-us engine op.
                        for rc in range(n_row_chunks):
                            a, b = rc * rows, rc * rows + rows
                            # hi = bf16(x)
                            nc.scalar.copy(hi[:, 1 + a:1 + b, 1:w + 1],
                                           xf[:, a:b, :])
                            # lo = bf16(x - hi)   (x - hi exact by Sterbenz)
                            nc.vector.tensor_sub(lo[:, 1 + a:1 + b, 1:w + 1],
                                                 xf[:, a:b, :],
                                                 hi[:, 1 + a:1 + b, 1:w + 1])
                        xpad[("lo", ki)] = lo
                    else:
                        nc.scalar.copy(hi[:, 1:h + 1, 1:w + 1], xf[:])
                        hif = hif_p.tile([128, h, w], F32, tag="hif")
                        nc.scalar.copy(hif[:], hi[:, 1:h + 1, 1:w + 1])
                        tmp = hif_p.tile([128, h, w], F32, tag="tmp")
                        nc.vector.tensor_sub(tmp[:], xf[:], hif[:])
                        fw = w + 1
                        body = lo8[:, ki, fw + 1:(h + 1) * fw + 1].rearrange(
                            "p (r c) -> p r c", c=fw)
                        nc.vector.tensor_scalar_mul(
                            body[:, :, 0:w], tmp[:], LO_SCALE)

                for rc in range(n_row_chunks):
                    for oc in range(OC):
                        r0 = rc * rows
                        ps = psum_p.tile([128, rows, w], F32)
                        k = 0
                        if mode == "bf16":
                            nmm = 2 * KI * KH * KW
                            for p in ("hi", "lo"):
                                for ki in range(KI):
                                    xp = xpad[(p, ki)]
                                    for ky in range(KH):
                                        for kx in range(KW):
                                            nc.tensor.matmul(
                                                ps[:],
                                                lhsT_all[:, lidx(ki, ky, kx, oc), :],
                                                xp[:, r0 + ky:r0 + ky + rows,
                                                   kx:kx + w],
                                                start=(k == 0),
                                                stop=(k == nmm - 1))
                                            k += 1
                            ob = out_p.tile([128, rows, w], F32)
                            nc.scalar.activation(
                                ob[:], ps[:],
                                mybir.ActivationFunctionType.Identity,
                                bias=bias_bin[:, oc:oc + 1], scale=1.0)
                        else:
                            nmm = KI * KH * KW
                            for ki in range(KI):
                                xp = xpad[("hi", ki)]
                                for ky in range(KH):
                                    for kx in range(KW):
                                        nc.tensor.matmul(
                                            ps[:],
                                            lhsT_all[:, lidx(ki, ky, kx, oc), :],
                                            xp[:, r0 + ky:r0 + ky + rows,
                                               kx:kx + w],
                                            start=(k == 0),
                                            stop=(k == nmm - 1))
                                        k += 1
                            fw = w + 1
                            psB = psumB_p.tile([128, rows * fw], F32)
                            for j2, (ky, kx) in enumerate(
                                    (a, b) for a in range(KH) for b in range(KW)):
                                j = (ky * KW + kx) * OC + oc
                                s = (r0 + ky) * fw + kx
                                nc.tensor.matmul(
                                    psB[:],
                                    lhsT8[:, j, :, :],
                                    lo8[:, :, s:s + rows * fw],
                                    start=(j2 == 0),
                                    stop=(j2 == KH * KW - 1),
                                    perf_mode=mybir.MatmulPerfMode.DoubleRow)
                            # combine: out = hi_psum + lo_psum/512 + bias
                            tmp_sb = out_p.tile([128, rows, w], F32, tag="cmb")
                            psBv = psB[:].rearrange("p (r c) -> p r c", c=fw)
                            nc.scalar.activation(
                                tmp_sb[:], psBv[:, :, 0:w],
                                mybir.ActivationFunctionType.Identity,
                                bias=bias_bin[:, oc:oc + 1], scale=1.0 / LO_SCALE)
                            ob = out_p.tile([128, rows, w], F32)
                            nc.vector.tensor_add(ob[:], tmp_sb[:], ps[:])
                        nc.sync.dma_start(
                            out=o_d[n, oc * 128:(oc + 1) * 128, r0:r0 + rows, :],
                            in_=ob[:])

    nc.compile()
    return nc


_CACHE = {}


def _get_program():
    if "nc" not in _CACHE:
        _CACHE["nc"] = build_program()
    return _CACHE["nc"]


def kernel(x, weight, bias):
    x = np.ascontiguousarray(x, dtype=np.float32)
    weight = np.ascontiguousarray(weight, dtype=np.float32)
    bias = np.ascontiguousarray(bias, dtype=np.float32)
    nc = _get_program()
    in_maps = [
        {"x": x[c * BPC:(c + 1) * BPC], "weight": weight, "bias": bias}
        for c in range(N_CORES)
    ]
    r = run_bass_kernel_spmd(nc, in_maps, list(range(N_CORES)))
    return np.concatenate([r.results[c]["out"] for c in range(N_CORES)], axis=0)



# revision 7
# speedup vs baseline: 1.0781x; 1.0781x over previous
"""Binary-weight 3x3 conv (BinaryConv2d) Trainium2 Bass kernel.

Reference computation (x[32,256,56,56] f32, w[256,256,3,3] f32, b[256] f32):
    out = conv2d(x, sign(w), pad=1) + sign(b)[None,:,None,None]

Strategy (v2 — Winograd):
  - Data-parallel over batch: 8 cores x 4 images each. No collectives.
  - F(4,3) Winograd along H, direct 3-tap accumulation along W:
      per output-channel chunk the PE does 6(k) x 3(kx) x 2(ki) matmuls per
      4-row band group instead of 9 x 2 — 4.5 MACs/output vs 9 (2x fewer
      PE cycles than the direct method; the direct f32r kernel measures
      451.6us/iter locally, PE-bound at 1 row/cycle).
  - All transforms in fp16: data B^T on DVE (contiguous row ops, 2x mode),
    weight G-combos exact-ish in fp16 (|err| ~2^-12 on 1/6, 1/24), inverse
    A^T on DVE in fp16 after ACT drains PSUM->SBUF (bias folded into the
    M1 drain since A^T column 1 is all-ones).
  - Output stored fp16 (host upcasts to f32); rel-err budget 2e-2 vs
    fp16-noise ~3e-4.
"""

from contextlib import ExitStack

import numpy as np

import concourse.bacc as bacc
import concourse.bass as bass
import concourse.tile as tile
import concourse.mybir as mybir
from concourse import masks
from concourse.bass_utils import run_bass_kernel_spmd

F32 = mybir.dt.float32
F16 = mybir.dt.float16

N_CORES = 8
B, C, H, W = 32, 256, 56, 56
O = 256
KH = KW = 3
BPC = B // N_CORES  # images per core
KI = C // 128       # input-channel chunks
OC = O // 128       # output-channel chunks

M = 4               # winograd output rows per tile: F(4,3)
T = M + 2           # transformed planes
NT = H // M         # tile-row bands per image (14)
NCH = 2             # band chunks per image for matmul/psum (7 bands each)
CB = NT // NCH      # bands per chunk (7)
FD = CB * W         # matmul free size (392)

AL = mybir.AluOpType


def build_program(bpc=BPC, h=H, w=W, repeat=1):
    """Build the per-core Bass program. Returns compiled nc."""
    assert h % M == 0
    nt = h // M
    cb = nt // NCH
    fd = cb * w
    pw = w + 2

    nc = bacc.Bacc("TRN2", target_bir_lowering=False, debug=False,
                   num_devices=N_CORES)
    x_d = nc.dram_tensor("x", [bpc, C, h, w], F32, kind="ExternalInput").ap()
    w_d = nc.dram_tensor("weight", [O, C, KH, KW], F32,
                         kind="ExternalInput").ap()
    b_d = nc.dram_tensor("bias", [O], F32, kind="ExternalInput").ap()
    o_d = nc.dram_tensor("out", [bpc, O, h, w], F16, kind="ExternalOutput").ap()

    with tile.TileContext(nc) as tc, ExitStack() as ctx:
        const = ctx.enter_context(tc.tile_pool(name="const", bufs=1))
        xstg_p = ctx.enter_context(tc.tile_pool(name="xstg", bufs=2))
        xpad_p = ctx.enter_context(tc.tile_pool(name="xpad", bufs=2))
        v_p = ctx.enter_context(tc.tile_pool(name="vp", bufs=2))
        s_p = ctx.enter_context(tc.tile_pool(name="sp", bufs=2))
        ms_p = ctx.enter_context(tc.tile_pool(name="msp", bufs=3))
        out_p = ctx.enter_context(tc.tile_pool(name="outp", bufs=4))

        # ---- constants ----
        identity = const.tile([128, 128], F16)
        masks.make_identity(nc, identity[:])

        bias_raw = const.tile([128, OC], F32)
        bias_bin = const.tile([128, OC], F32)
        nc.sync.dma_start(out=bias_raw[:],
                          in_=b_d.rearrange("(b a) -> a b", b=OC))
        nc.scalar.sign(bias_bin[:], bias_raw[:])

        # ---- weights: load, binarize, transpose, G-combine along ky ----
        # lhsT_raw[:, idx, :] = sign(W[.., ky, kx]).T  with
        #   idx = ((ky*KW + kx)*KI + ki)*OC + oc      (ky-major blocks)
        # lhsT_U[:, idxu, :] = U_k[kx, ki, oc]  with
        #   idxu = ((k*KW + kx)*KI + ki)*OC + oc      (k-major blocks)
        NTAP = KI * OC * KH * KW

        def idx_raw(ky, kx, ki, oc):
            return ((ky * KW + kx) * KI + ki) * OC + oc

        def idx_u(k, kx, ki, oc):
            return ((k * KW + kx) * KI + ki) * OC + oc

        lhsT_U = const.tile([128, T * KW * KI * OC, 128], F16)

        wstg_ctx = ExitStack()
        wstg_p = wstg_ctx.enter_context(tc.tile_pool(name="wstg", bufs=2))
        tpsum_p = wstg_ctx.enter_context(
            tc.tile_pool(name="tpsum", bufs=2, space=bass.MemorySpace.PSUM))
        lhsT_raw = wstg_p.tile([128, KH * KW * KI * OC, 128], F16, tag="raw",
                               bufs=1)
        for ki in range(KI):
            for oc in range(OC):
                wstg = wstg_p.tile([128, 128, KH, KW], F32, tag="wstg")
                nc.sync.dma_start(
                    out=wstg[:],
                    in_=w_d[oc * 128:(oc + 1) * 128,
                            ki * 128:(ki + 1) * 128, :, :])
                wbin = wstg_p.tile([128, 128, KH, KW], F16, tag="wbin",
                                   bufs=1)
                nc.scalar.sign(wbin[:], wstg[:])
                for ky in range(KH):
                    for kx in range(KW):
                        tp = tpsum_p.tile([128, 128], F16)
                        nc.tensor.transpose(tp[:], wbin[:, :, ky, kx],
                                            identity[:])
                        nc.vector.tensor_copy(
                            lhsT_raw[:, idx_raw(ky, kx, ki, oc), :], tp[:])

        # G = [[1/4,0,0],[-1/6,-1/6,-1/6],[-1/6,1/6,-1/6],
        #      [1/24,1/12,1/6],[1/24,-1/12,1/6],[0,0,1]]
        NB = KW * KI * OC  # tiles per ky/k block (12)

        def rawb(ky):
            return lhsT_raw[:, ky * NB:(ky + 1) * NB, :]

        def ub(k):
            return lhsT_U[:, k * NB:(k + 1) * NB, :]

        g0, g1, g2 = rawb(0), rawb(1), rawb(2)
        wt1 = wstg_p.tile([128, NB, 128], F16, tag="wt1", bufs=1)
        wt2 = wstg_p.tile([128, NB, 128], F16, tag="wt2", bufs=1)
        # U0 = g0/4 ; U5 = g2
        nc.vector.tensor_scalar_mul(ub(0), g0, 0.25)
        nc.vector.tensor_copy(ub(5), g2)
        # U1 = -(g0+g1+g2)/6 ; U2 = (g1-g0-g2)/6
        nc.vector.tensor_add(wt1[:], g0, g2)           # t1 = g0+g2
        nc.vector.tensor_add(wt2[:], wt1[:], g1)
        nc.vector.tensor_scalar_mul(ub(1), wt2[:], -1.0 / 6.0)
        nc.vector.tensor_sub(wt2[:], g1, wt1[:])
        nc.vector.tensor_scalar_mul(ub(2), wt2[:], 1.0 / 6.0)
        # U3 = (g0+2g1+4g2)/24 ; U4 = (g0-2g1+4g2)/24
        nc.vector.scalar_tensor_tensor(wt1[:], g1, 2.0, g0,
                                       op0=AL.mult, op1=AL.add)
        nc.vector.scalar_tensor_tensor(wt2[:], g2, 4.0, wt1[:],
                                       op0=AL.mult, op1=AL.add)
        nc.vector.tensor_scalar_mul(ub(3), wt2[:], 1.0 / 24.0)
        nc.vector.scalar_tensor_tensor(wt1[:], g1, -2.0, g0,
                                       op0=AL.mult, op1=AL.add)
        nc.vector.scalar_tensor_tensor(wt2[:], g2, 4.0, wt1[:],
                                       op0=AL.mult, op1=AL.add)
        nc.vector.tensor_scalar_mul(ub(4), wt2[:], 1.0 / 24.0)
        wstg_ctx.close()

        psum_p = ctx.enter_context(
            tc.tile_pool(name="psum", bufs=8, space=bass.MemorySpace.PSUM))

        # ---- main loop over images ----
        for _rep in range(repeat):
            for n in range(bpc):
                V = {}
                for ki in range(KI):
                    xf = xstg_p.tile([128, h, w], F32, tag="xf")
                    hh = h // 2
                    nc.sync.dma_start(
                        out=xf[:, :hh, :],
                        in_=x_d[n, ki * 128:(ki + 1) * 128, :hh, :])
                    nc.sync.dma_start(
                        out=xf[:, hh:, :],
                        in_=x_d[n, ki * 128:(ki + 1) * 128, hh:, :])
                    # padded fp16 image; rows 0..57 used (58,59 dead pad to
                    # make the row count divisible by 4 for the band view)
                    xp = xpad_p.tile([128, 60, pw], F16, tag=f"xp{ki}")
                    nc.gpsimd.memset(xp[:, 0, :], 0.0)
                    nc.gpsimd.memset(xp[:, h + 1, :], 0.0)
                    nc.gpsimd.memset(xp[:, 1:h + 1, 0:1], 0.0)
                    nc.gpsimd.memset(xp[:, 1:h + 1, w + 1:pw], 0.0)
                    nc.scalar.copy(xp[:, 1:1 + hh, 1:w + 1], xf[:, :hh, :])
                    nc.scalar.copy(xp[:, 1 + hh:1 + h, 1:w + 1], xf[:, hh:, :])

                    # forward transform: V[k][:, r', :] over bands of 4 rows
                    # d_j = xp[4r' + j], j = 0..5
                    xpv = xp[:].rearrange("p (r q) c -> p q r c", q=4)

                    def dj(j):
                        return xpv[:, j % 4, j // 4:j // 4 + nt, :]

                    vt = v_p.tile([128, T, nt, pw], F16, tag=f"V{ki}")

                    def sv(name):
                        return s_p.tile([128, nt, pw], F16, tag=name,
                                        name=name)

                    stt = nc.vector.scalar_tensor_tensor
                    d0, d1, d2, d3, d4, d5 = (dj(j) for j in range(6))
                    s1, s2, s3, s4 = sv("s1"), sv("s2"), sv("s3"), sv("s4")
                    s5, s6, s7, s8 = sv("s5"), sv("s6"), sv("s7"), sv("s8")
                    # V0 = 4d0 - 5d2 + d4      = -4(d2-d0) + (d4-d2)
                    # V1 = -4d1 - 4d2 + d3 + d4 = -4(d1+d2) + (d3+d4)
                    # V2 = 4d1 - 4d2 - d3 + d4  =  4(d1-d2) + (d4-d3)
                    # V3 = -2d1 - d2 + 2d3 + d4 =  2(d3-d1) + (d4-d2)
                    # V4 = 2d1 - d2 - 2d3 + d4  = -2(d3-d1) + (d4-d2)
                    # V5 = 4d1 - 5d3 + d5       = -4(d3-d1) + (d5-d3)
                    nc.vector.tensor_add(s2[:], d1, d2)
                    nc.vector.tensor_add(s1[:], d3, d4)
                    stt(vt[:, 1], s2[:], -4.0, s1[:], op0=AL.mult, op1=AL.add)
                    nc.vector.tensor_sub(s3[:], d1, d2)
                    nc.vector.tensor_sub(s4[:], d4, d3)
                    stt(vt[:, 2], s3[:], 4.0, s4[:], op0=AL.mult, op1=AL.add)
                    nc.vector.tensor_sub(s5[:], d4, d2)
                    nc.vector.tensor_sub(s6[:], d3, d1)
                    stt(vt[:, 3], s6[:], 2.0, s5[:], op0=AL.mult, op1=AL.add)
                    stt(vt[:, 4], s6[:], -2.0, s5[:], op0=AL.mult, op1=AL.add)
                    nc.vector.tensor_sub(s7[:], d2, d0)
                    stt(vt[:, 0], s7[:], -4.0, s5[:], op0=AL.mult, op1=AL.add)
                    nc.vector.tensor_sub(s8[:], d5, d3)
                    stt(vt[:, 5], s6[:], -4.0, s8[:], op0=AL.mult, op1=AL.add)
                    V[ki] = vt

                for oc in range(OC):
                    for ch in range(NCH):
                        r0 = ch * cb
                        ms = []
                        for k in range(T):
                            ps = psum_p.tile([128, cb, w], F32)
                            mm = 0
                            for ki in range(KI):
                                for kx in range(KW):
                                    nc.tensor.matmul(
                                        ps[:],
                                        lhsT_U[:, idx_u(k, kx, ki, oc), :],
                                        V[ki][:, k, r0:r0 + cb, kx:kx + w],
                                        start=(mm == 0),
                                        stop=(mm == KI * KW - 1))
                                    mm += 1
                            # drain to fp16 SBUF (bias folds into M1: A^T
                            # column 1 is all-ones)
                            mk = ms_p.tile([128, cb, w], F16, tag=f"m{k}")
                            if k == 1:
                                nc.scalar.activation(
                                    mk[:], ps[:],
                                    mybir.ActivationFunctionType.Identity,
                                    bias=bias_bin[:, oc:oc + 1], scale=1.0)
                            else:
                                nc.scalar.copy(mk[:], ps[:])
                            ms.append(mk)

                        # inverse transform A^T (fp16, DVE)
                        # o0 = M0+M1+M2+M3+M4 ; o1 = M1-M2+2(M3-M4)
                        # o2 = M1+M2+4(M3+M4) ; o3 = M1-M2+8(M3-M4)+M5
                        ob = out_p.tile([128, M * cb, w], F16, tag="ob")
                        obv = ob[:].rearrange("p (r q) c -> p q r c", q=M)
                        P = s_p.tile([128, cb, w], F16, tag="iP")
                        Q = s_p.tile([128, cb, w], F16, tag="iQ")
                        R = s_p.tile([128, cb, w], F16, tag="iR")
                        S = s_p.tile([128, cb, w], F16, tag="iS")
                        t0 = s_p.tile([128, cb, w], F16, tag="it0")
                        stt = nc.vector.scalar_tensor_tensor
                        nc.vector.tensor_add(P[:], ms[1][:], ms[2][:])
                        nc.vector.tensor_sub(Q[:], ms[1][:], ms[2][:])
                        nc.vector.tensor_add(R[:], ms[3][:], ms[4][:])
                        nc.vector.tensor_sub(S[:], ms[3][:], ms[4][:])
                        nc.vector.tensor_add(t0[:], ms[0][:], R[:])
                        nc.vector.tensor_add(obv[:, 0], t0[:], P[:])
                        stt(obv[:, 1], S[:], 2.0, Q[:],
                            op0=AL.mult, op1=AL.add)
                        stt(obv[:, 2], R[:], 4.0, P[:],
                            op0=AL.mult, op1=AL.add)
                        nc.vector.tensor_add(t0[:], Q[:], ms[5][:])
                        stt(obv[:, 3], S[:], 8.0, t0[:],
                            op0=AL.mult, op1=AL.add)
                        nc.sync.dma_start(
                            out=o_d[n, oc * 128:(oc + 1) * 128,
                                    ch * M * cb:(ch + 1) * M * cb, :],
                            in_=ob[:])

    nc.compile()
    return nc


_CACHE = {}


def _get_program():
    if "nc" not in _CACHE:
        _CACHE["nc"] = build_program()
    return _CACHE["nc"]


def kernel(x, weight, bias):
    x = np.ascontiguousarray(x, dtype=np.float32)
    weight = np.ascontiguousarray(weight, dtype=np.float32)
    bias = np.ascontiguousarray(bias, dtype=np.float32)
    nc = _get_program()
    in_maps = [
        {"x": x[c * BPC:(c + 1) * BPC], "weight": weight, "bias": bias}
        for c in range(N_CORES)
    ]
    r = run_bass_kernel_spmd(nc, in_maps, list(range(N_CORES)))
    return np.concatenate(
        [r.results[c]["out"].astype(np.float32) for c in range(N_CORES)],
        axis=0)


# revision 13
# speedup vs baseline: 1.0785x; 1.0004x over previous
"""Binary-weight 3x3 conv (BinaryConv2d) Trainium2 Bass kernel.

Reference computation (x[32,256,56,56] f32, w[256,256,3,3] f32, b[256] f32):
    out = conv2d(x, sign(w), pad=1) + sign(b)[None,:,None,None]

Strategy (v2 — F(4,3) Winograd along H, direct 3-tap along W):
  - Data-parallel over batch: 8 cores x 4 images each. No collectives.
  - PE does 6(k-plane) x 3(kx) x 2(ki) matmuls per 7-band chunk instead of
    9 x 2 direct taps: 4.5 MACs/output vs 9 (2x fewer PE row-cycles; the
    direct kernel is PE-bound at 451.6us/iter locally).
  - The local backend charges ~0.6-1.5us fixed cost per vector-engine op,
    so transforms are organized as few, wide "mega-ops": strided
    multi-component access patterns compute up to 4 subexpressions per
    instruction. Forward transform: 10 vector ops per (ki, image) via a
    packed subexpression tile; V-plane order is permuted (and U2 negated)
    so combine steps pair into affine 2-component ops.
  - Work split: GPSIMD runs the forward transform (SBUF-only ops), DVE
    runs the inverse (PSUM reads are DVE-only), ACT only does the
    f32->fp16 padded-image copy, bias rides the PE as a K=1 ones-matmul
    accumulated into the M1-plane psum group (A^T column 1 is all-ones).
  - Everything fp16 (exact enough: rel err ~5e-3 << 2e-2 gate), output
    stored fp16 and upcast on host.
"""

from contextlib import ExitStack

import numpy as np

import concourse.bacc as bacc
import concourse.bass as bass
import concourse.tile as tile
import concourse.mybir as mybir
from concourse import masks
from concourse.bass_utils import run_bass_kernel_spmd

F32 = mybir.dt.float32
F16 = mybir.dt.float16

N_CORES = 8
B, C, H, W = 32, 256, 56, 56
O = 256
KH = KW = 3
BPC = B // N_CORES  # images per core
KI = C // 128       # input-channel chunks
OC = O // 128       # output-channel chunks

M = 4               # winograd output rows per tile: F(4,3)
T = M + 2           # transformed planes
NT = H // M         # tile-row bands per image (14)
NCH = 2             # band chunks per image for matmul/psum (7 bands each)

AL = mybir.AluOpType

# V-plane position -> original winograd k index (U2 carries a flipped sign:
# position 5 holds -V2, compensated by negating U_2 and swapping P/Q roles).
#   pos: [V0, V5, V3, V4, V1, -V2]
POS_K = [0, 5, 3, 4, 1, 2]


def build_program(bpc=BPC, h=H, w=W, repeat=1):
    """Build the per-core Bass program. Returns compiled nc."""
    assert h % M == 0
    nt = h // M
    cb = nt // NCH          # bands per chunk (7)
    fd = cb * w             # matmul free size (392)
    pw = w + 2              # V width with conv column pads

    nc = bacc.Bacc("TRN2", target_bir_lowering=False, debug=False,
                   num_devices=N_CORES)
    x_d = nc.dram_tensor("x", [bpc, C, h, w], F32, kind="ExternalInput").ap()
    w_d = nc.dram_tensor("weight", [O, C, KH, KW], F32,
                         kind="ExternalInput").ap()
    b_d = nc.dram_tensor("bias", [O], F32, kind="ExternalInput").ap()
    o_d = nc.dram_tensor("out", [bpc, O, h, w], F16, kind="ExternalOutput").ap()

    with tile.TileContext(nc) as tc, ExitStack() as ctx:
        const = ctx.enter_context(tc.tile_pool(name="const", bufs=1))
        xstg_p = ctx.enter_context(tc.tile_pool(name="xstg", bufs=2))
        xpad_p = ctx.enter_context(tc.tile_pool(name="xpad", bufs=2))
        v_p = ctx.enter_context(tc.tile_pool(name="vp", bufs=2))
        s_p = ctx.enter_context(tc.tile_pool(name="sp", bufs=2))
        out_p = ctx.enter_context(tc.tile_pool(name="outp", bufs=4))

        # ---- constants ----
        identity = const.tile([128, 128], F16)
        masks.make_identity(nc, identity[:])

        ones_row = const.tile([1, 512], F16)
        nc.gpsimd.memset(ones_row[:], 1.0)
        b_raw = const.tile([1, O], F32)
        nc.sync.dma_start(out=b_raw[:], in_=b_d.rearrange("(a b) -> a b", a=1))
        b_row = const.tile([1, O], F16)
        nc.scalar.sign(b_row[:], b_raw[:])

        # ---- weights: load, binarize, transpose, G-combine along ky ----
        # lhsT_U[:, idxu, :] = U'_p[kx, ki, oc] with p the V position order;
        #   idxu = ((p*KW + kx)*KI + ki)*OC + oc
        # U'_p = G-combo for k=POS_K[p], negated for p=5.
        NB = KW * KI * OC  # tiles per k block (12)

        def idx_raw(ky, kx, ki, oc):
            return ((ky * KW + kx) * KI + ki) * OC + oc

        lhsT_U = const.tile([128, T * NB, 128], F16)

        wstg_ctx = ExitStack()
        wstg_p = wstg_ctx.enter_context(tc.tile_pool(name="wstg", bufs=2))
        tpsum_p = wstg_ctx.enter_context(
            tc.tile_pool(name="tpsum", bufs=2, space=bass.MemorySpace.PSUM))
        lhsT_raw = wstg_p.tile([128, KH * KW * KI * OC, 128], F16, tag="raw",
                               bufs=1)
        for ki in range(KI):
            for oc in range(OC):
                wstg = wstg_p.tile([128, 128, KH, KW], F32, tag="wstg")
                nc.sync.dma_start(
                    out=wstg[:],
                    in_=w_d[oc * 128:(oc + 1) * 128,
                            ki * 128:(ki + 1) * 128, :, :])
                wbin = wstg_p.tile([128, 128, KH, KW], F16, tag="wbin",
                                   bufs=1)
                nc.scalar.sign(wbin[:], wstg[:])
                for ky in range(KH):
                    for kx in range(KW):
                        tp = tpsum_p.tile([128, 128], F16)
                        nc.tensor.transpose(tp[:], wbin[:, :, ky, kx],
                                            identity[:])
                        nc.vector.tensor_copy(
                            lhsT_raw[:, idx_raw(ky, kx, ki, oc), :], tp[:])

        # G rows (k): U0=g0/4, U1=-(g0+g1+g2)/6, U2=(g1-g0-g2)/6,
        #             U3=(g0+2g1+4g2)/24, U4=(g0-2g1+4g2)/24, U5=g2
        def rawb(ky):
            return lhsT_raw[:, ky * NB:(ky + 1) * NB, :]

        def ub(k_pos):
            return lhsT_U[:, k_pos * NB:(k_pos + 1) * NB, :]

        # position mapping: ub(pos) gets U_{POS_K[pos]} (neg for pos 5)
        UPOS = {k: p for p, k in enumerate(POS_K)}
        g0, g1, g2 = rawb(0), rawb(1), rawb(2)
        wt1 = wstg_p.tile([128, NB, 128], F16, tag="wt1", bufs=1)
        wt2 = wstg_p.tile([128, NB, 128], F16, tag="wt2", bufs=1)
        nc.vector.tensor_scalar_mul(ub(UPOS[0]), g0, 0.25)
        nc.vector.tensor_copy(ub(UPOS[5]), g2)
        nc.vector.tensor_add(wt1[:], g0, g2)
        nc.vector.tensor_add(wt2[:], wt1[:], g1)
        nc.vector.tensor_scalar_mul(ub(UPOS[1]), wt2[:], -1.0 / 6.0)
        nc.vector.tensor_sub(wt2[:], g1, wt1[:])
        # +U2 here: position 5 stores -V2 data, so M_pos5 = U2 * (-V2) = -M2
        nc.vector.tensor_scalar_mul(ub(UPOS[2]), wt2[:], 1.0 / 6.0)
        nc.vector.tensor_add(wt1[:], g1, g1)
        nc.vector.tensor_add(wt2[:], wt1[:], g0)      # g0+2g1
        nc.vector.tensor_add(wt1[:], g2, g2)
        nc.vector.tensor_add(wt1[:], wt1[:], wt1[:])  # 4g2
        nc.vector.tensor_add(wt2[:], wt2[:], wt1[:])  # g0+2g1+4g2
        nc.vector.tensor_scalar_mul(ub(UPOS[3]), wt2[:], 1.0 / 24.0)
        nc.vector.tensor_sub(wt2[:], g0, g1)
        nc.vector.tensor_sub(wt2[:], wt2[:], g1)      # g0-2g1
        nc.vector.tensor_add(wt2[:], wt2[:], wt1[:])  # g0-2g1+4g2
        nc.vector.tensor_scalar_mul(ub(UPOS[4]), wt2[:], 1.0 / 24.0)
        wstg_ctx.close()

        psum_p = ctx.enter_context(
            tc.tile_pool(name="psum", bufs=8, space=bass.MemorySpace.PSUM))

        # ---- main loop over images ----
        for _rep in range(repeat):
            for n in range(bpc):
                V = {}
                for ki in range(KI):
                    # f32 row-padded staging (rows 1..56 data, 0 and 57 zero)
                    xfp = xstg_p.tile([128, h + 2, w], F32, tag="xfp")
                    hh = h // 2
                    nc.sync.dma_start(
                        out=xfp[:, 1:1 + hh, :],
                        in_=x_d[n, ki * 128:(ki + 1) * 128, :hh, :])
                    nc.sync.dma_start(
                        out=xfp[:, 1 + hh:1 + h, :],
                        in_=x_d[n, ki * 128:(ki + 1) * 128, hh:, :])
                    nc.gpsimd.memset(xfp[:, 0, :], 0.0)
                    nc.gpsimd.memset(xfp[:, h + 1, :], 0.0)
                    # fp16 copy (ACT), 60 rows so the q=4 band view divides
                    xp = xpad_p.tile([128, h + 4, w], F16, tag=f"xp{ki}")
                    nc.scalar.copy(xp[:, 0:h + 2, :], xfp[:])

                    # subexpression mega-ops.
                    # S components: 0:s7=d2-d0 1:s6=d3-d1 2:s5=d4-d2
                    #   3:s8=d5-d3 4:s2=d1+d2 5:s1=d3+d4 6:s3=d1-d2
                    #   7:s4'=d3-d4     (d_j = padded row 4r'+j)
                    st = s_p.tile([128, 8, nt, w], F16, tag="st")
                    hi = xp[:, 2:2 + 4 * nt, :].rearrange(
                        "p (r q) c -> p q r c", q=4)
                    lo = xp[:, 0:4 * nt, :].rearrange(
                        "p (r q) c -> p q r c", q=4)
                    nc.gpsimd.tensor_sub(st[:, 0:4], hi, lo)
                    odd = xp[:, 1:1 + 4 * nt, :].rearrange(
                        "p (r b t) c -> p t b r c", b=2, t=2)
                    nc.gpsimd.tensor_add(st[:, 4:6], odd[:, 0], odd[:, 1])
                    nc.gpsimd.tensor_sub(st[:, 6:8], odd[:, 0], odd[:, 1])

                    # scales via add-chains (GPS tensor_scalar is slow):
                    # tt: 0:4*s7 1:4*s6 2:4*s2 3:4*s3 4:2*s6
                    tt = s_p.tile([128, 5, nt, w], F16, tag="tt")
                    stv = st[:].rearrange("p s r c -> p s (r c)")
                    ttv = tt[:].rearrange("p s r c -> p s (r c)")
                    nc.gpsimd.tensor_add(ttv[:, 4], stv[:, 1], stv[:, 1])
                    nc.gpsimd.tensor_add(tt[:, 0:2], st[:, 0:2], st[:, 0:2])
                    nc.gpsimd.tensor_add(tt[:, 0:2], tt[:, 0:2], tt[:, 0:2])
                    s23 = st[:].rearrange("p (a s) r c -> p s a r c", s=2)
                    nc.gpsimd.tensor_add(tt[:, 2:4], s23[:, 0, 2:4],
                                         s23[:, 0, 2:4])
                    nc.gpsimd.tensor_add(tt[:, 2:4], tt[:, 2:4], tt[:, 2:4])

                    # V combines into position-ordered planes, cols 1..56
                    vt = v_p.tile([128, T, nt, pw], F16, tag=f"V{ki}")
                    nc.gpsimd.memset(vt[:, :, :, 0], 0.0)
                    nc.gpsimd.memset(vt[:, :, :, pw - 1], 0.0)
                    vin = vt[:, :, :, 1:w + 1]
                    # pos0 = V0 = s5-4s7 ; pos1 = V5 = s8-4s6
                    nc.gpsimd.tensor_sub(vin[:, 0:2], st[:, 2:4], tt[:, 0:2])
                    # pos2 = V3 = s5+2s6 ; pos3 = V4 = s5-2s6
                    nc.gpsimd.tensor_add(vin[:, 2], st[:, 2], tt[:, 4])
                    nc.gpsimd.tensor_sub(vin[:, 3], st[:, 2], tt[:, 4])
                    # pos4 = V1 = s1-4s2 ; pos5 = -V2 = s4'-4s3
                    s17 = st[:].rearrange("p (a s) r c -> p s a r c", s=2)
                    nc.gpsimd.tensor_sub(vin[:, 4:6], s17[:, 1, 2:4],
                                         tt[:, 2:4])
                    V[ki] = vt

                for oc in range(OC):
                    for ch in range(NCH):
                        r0 = ch * cb
                        ms = []
                        for p in range(T):
                            ps = psum_p.tile([128, cb, w], F32)
                            mm = 0
                            if p == 4:  # M1 group opens with the bias matmul
                                nc.tensor.matmul(
                                    ps[:],
                                    b_row[:, oc * 128:(oc + 1) * 128],
                                    ones_row[:, :fd].rearrange(
                                        "a (r c) -> a r c", c=w),
                                    start=True, stop=False)
                                mm = 1
                            for ki in range(KI):
                                for kx in range(KW):
                                    nc.tensor.matmul(
                                        ps[:],
                                        lhsT_U[:, ((p * KW + kx) * KI + ki)
                                               * OC + oc, :],
                                        V[ki][:, p, r0:r0 + cb, kx:kx + w],
                                        start=(mm == 0),
                                        stop=(mm == (KI * KW
                                                     + (1 if p == 4 else 0)
                                                     - 1)))
                                    mm += 1
                            ms.append(ps)

                        # inverse A^T on DVE (psum planes in position order:
                        # p0=M0 p1=M5 p2=M3 p3=M4 p4=M1(+bias) p5=-M2)
                        ob = out_p.tile([128, M * cb, w], F16, tag="ob")
                        obv = ob[:].rearrange("p (r q) c -> p q r c", q=M)
                        iP = s_p.tile([128, cb, w], F16, tag="iP")
                        iQ = s_p.tile([128, cb, w], F16, tag="iQ")
                        iR = s_p.tile([128, cb, w], F16, tag="iR")
                        iS = s_p.tile([128, cb, w], F16, tag="iS")
                        it = s_p.tile([128, cb, w], F16, tag="it")
                        # DVE may read only one PSUM operand per op: ACT
                        # drains one plane of each +/- pair to fp16 SBUF.
                        m5s = s_p.tile([128, cb, w], F16, tag="m5s")
                        m3s = s_p.tile([128, cb, w], F16, tag="m3s")
                        nc.scalar.copy(m5s[:], ms[5][:])
                        nc.scalar.copy(m3s[:], ms[3][:])
                        # P=M1+M2=p4-p5  Q=M1-M2=p4+p5  R=M3+M4  S=M3-M4
                        nc.vector.tensor_sub(iP[:], ms[4][:], m5s[:])
                        nc.vector.tensor_add(iQ[:], ms[4][:], m5s[:])
                        nc.vector.tensor_add(iR[:], ms[2][:], m3s[:])
                        nc.vector.tensor_sub(iS[:], ms[2][:], m3s[:])
                        # o0 = M0 + P + R
                        nc.vector.tensor_add(it[:], ms[0][:], iR[:])
                        nc.vector.tensor_add(obv[:, 0], it[:], iP[:])
                        # o1 = Q + 2S
                        nc.vector.tensor_scalar_mul(it[:], iS[:], 2.0)
                        nc.vector.tensor_add(obv[:, 1], iQ[:], it[:])
                        # o2 = P + 4R
                        nc.vector.tensor_scalar_mul(it[:], iR[:], 4.0)
                        nc.vector.tensor_add(obv[:, 2], iP[:], it[:])
                        # o3 = Q + 8S + M5
                        nc.vector.tensor_scalar_mul(it[:], iS[:], 8.0)
                        nc.vector.tensor_add(it[:], iQ[:], it[:])
                        nc.vector.tensor_add(obv[:, 3], it[:], ms[1][:])
                        nc.sync.dma_start(
                            out=o_d[n, oc * 128:(oc + 1) * 128,
                                    ch * M * cb:(ch + 1) * M * cb, :],
                            in_=ob[:])

    nc.compile()
    return nc


_CACHE = {}


def _get_program():
    if "nc" not in _CACHE:
        _CACHE["nc"] = build_program()
    return _CACHE["nc"]


def kernel(x, weight, bias):
    x = np.ascontiguousarray(x, dtype=np.float32)
    weight = np.ascontiguousarray(weight, dtype=np.float32)
    bias = np.ascontiguousarray(bias, dtype=np.float32)
    nc = _get_program()
    in_maps = [
        {"x": x[c * BPC:(c + 1) * BPC], "weight": weight, "bias": bias}
        for c in range(N_CORES)
    ]
    r = run_bass_kernel_spmd(nc, in_maps, list(range(N_CORES)))
    return np.concatenate(
        [r.results[c]["out"].astype(np.float32) for c in range(N_CORES)],
        axis=0)
